# revision 12
# baseline (speedup 1.0000x reference)
"""MinGRU layer (LN -> gate/candidate Linear -> minGRU scan -> residual) on 8 trn2 cores.

Problem (hardcoded): x [B=4, T=4096, H=1024] fp32, weights Wg/Wc [1024,1024],
biases bg/bc [1024], LN gamma/beta [1024].

Sharding: core c = (batch b = c//2, output-half p = c%2). Every core receives
the full transposed batch row xT[b] = x[b].T (H on partitions, T on free) and
computes z/c for its 512 output channels over all T. The minGRU recurrence is
elementwise over (b, h), so with output-channel sharding each core scans its
own channels over the full sequence - no cross-core dependency, no collectives.

Per-core pipeline (layouts [h or o on partitions, t on free], 512-col chunks,
stats for chunk i+1 software-pipelined under the GEMMs of chunk i):
  1. LN folded algebraically. The mean-subtraction folds EXACTLY into the
     weights on host: sum_h A[o,h](x[h]-mu) = sum_h (A[o,h]-rowmean(A)[o])x[h]
     since mu is a multiple of sum_h x. gamma/beta fold into W''/b_eff as
     usual. So the device only needs rstd[t]: xn = x * rstdB, GEMM with W''.
     (This removes the K=1 "aug" matmuls of the previous version.)
  2. Stats: host ships a packed fp8 tensor [x; x^2] (e4m3). Per chunk, 8
     DoubleRow fp8 matmuls with a [1,0;0,1]-structured lhsT reduce BOTH
     sum(x) and sum(x^2) over all 1024 h rows in one PSUM tile (2 used rows).
     fp8 stats noise -> rstd rel err ~3e-4 RMS, ~3e-3 max: negligible vs the
     2e-2 gate. No on-device squares (frees Scalar+GpSimd).
  3. rstd without Ln/Exp (so Sigmoid/Square/Identity share ONE act table,
     zero ACT_TABLE_LOAD switches): seed y0 = (s*v+b)^2 via one Square ACT
     (max rel err 2.2e-2 on var+eps in [0.70,1.34]) then one Newton step
     y1 = y0*(1.5 - 0.5*v*y0^2) on DVE -> 7e-4 fit err.
  4. GEMMs in bf16 (fp32 PSUM), 2 x 4 o-tiles x 8 k matmuls per chunk.
  5. z = sigmoid(pre+bg) bf16; a = 1-z as sigmoid(-pre-bg); cpb = pc+bc on
     ScalarE (Identity+bias); b = cpb*z on DVE bf16.
  6. h = tensor_tensor_scan(a, b) on VectorE in bf16 (fp32 internal state),
     chained across chunks.
  7. out = h + x (bf16 residual read straight from the GEMM x tile - the
     host row-roll puts each core's own channels in k-tiles 0..3), computed
     on GpSimd, fp32 out; DMA out; host transposes shards back.
"""

import functools
import os
import numpy as np
import ml_dtypes

import concourse.bass as bass
import concourse.bacc as bacc
import concourse.tile as tile
import concourse.hw_specs as hw_specs
from concourse import mybir
from concourse.bass_utils import run_bass_kernel_spmd

# The table-load pass assigns each activation the FIRST act_func_set that
# contains it. We only use Sigmoid/Square/Copy/Identity, all present in
# sigmoid_and_others - but Square/Copy/Identity also appear in earlier sets,
# which would force table switches. Strip our funcs from every other set so
# all four resolve to sigmoid_and_others: ONE table load for the whole kernel.
_orig_get_act_tables = hw_specs.get_activation_tables
_OURS = {
    mybir.ActivationFunctionType.Sigmoid,
    mybir.ActivationFunctionType.Square,
    mybir.ActivationFunctionType.Copy,
    mybir.ActivationFunctionType.Identity,
}


@functools.cache
def _patched_get_act_tables(module_arch):
    d = dict(_orig_get_act_tables(module_arch))
    for name in d:
        if name != "sigmoid_and_others":
            d[name] = d[name] - _OURS
    return d


hw_specs.get_activation_tables = _patched_get_act_tables
bacc.get_activation_tables = _patched_get_act_tables

B, T, H = 4, 4096, 1024
EPS = 1e-5
N_CORES = 8
OH = H // 2          # output channels per core
CHUNK = 512
N_CHUNKS = T // CHUNK
KT = H // 128        # k-tiles (contraction)
OT = OH // 128       # o-tiles per core

F32 = mybir.dt.float32
BF16 = mybir.dt.bfloat16
F8 = mybir.dt.float8e4
AF = mybir.ActivationFunctionType
OP = mybir.AluOpType
PM = mybir.MatmulPerfMode
BF = ml_dtypes.bfloat16
NP8 = ml_dtypes.float8_e4m3

# rsqrt Newton seed: y0 = (S_SEED*v + B_SEED)^2 fit on v in [0.70, 1.34]
# (var+eps of N(0,1) rows with H=1024 is 1 +- ~0.045), then one Newton step.
S_SEED = -0.253250
B_SEED = 1.258673 + S_SEED * EPS

_CACHE = {}


def _build():
    nc = bacc.Bacc("TRN2", target_bir_lowering=False, debug=False)

    # all tensors host-pre-tiled so every DMA is fully contiguous
    xT_d = nc.dram_tensor("xT", [N_CHUNKS, 128, KT, CHUNK], BF16, kind="ExternalInput").ap()
    xs_d = nc.dram_tensor("xs", [N_CHUNKS, 128, KT, 2, CHUNK], F8, kind="ExternalInput").ap()
    wg_d = nc.dram_tensor("wg", [128, KT, OH], BF16, kind="ExternalInput").ap()
    wc_d = nc.dram_tensor("wc", [128, KT, OH], BF16, kind="ExternalInput").ap()
    bg_d = nc.dram_tensor("bg", [128, OT], F32, kind="ExternalInput").ap()
    bgn_d = nc.dram_tensor("bgn", [128, OT], F32, kind="ExternalInput").ap()
    bc_d = nc.dram_tensor("bc", [128, OT], F32, kind="ExternalInput").ap()
    ones2_d = nc.dram_tensor("ones2", [128, 2, 64], F8, kind="ExternalInput").ap()
    onesr_d = nc.dram_tensor("onesr", [1, 128], BF16, kind="ExternalInput").ap()
    ones1_d = nc.dram_tensor("ones1", [128, 1], BF16, kind="ExternalInput").ap()
    cst_d = nc.dram_tensor("cst", [1, 1], F32, kind="ExternalInput").ap()
    out_d = nc.dram_tensor("outT", [N_CHUNKS, OT, 128, CHUNK], F32, kind="ExternalOutput").ap()

    with tile.TileContext(nc) as tc:
        with (
            tc.tile_pool(name="const", bufs=1) as cpool,
            tc.tile_pool(name="xin", bufs=3) as xpool,
            tc.tile_pool(name="xst", bufs=3) as stpool,
            tc.tile_pool(name="xnp", bufs=2) as xnpool,
            tc.tile_pool(name="stat", bufs=2) as spool,
            tc.tile_pool(name="work", bufs=3) as wpool,
            tc.tile_pool(name="hbuf", bufs=2) as hpool,
            tc.tile_pool(name="psA", bufs=3, space="PSUM") as psA,
            tc.tile_pool(name="psB", bufs=2, space="PSUM") as psB,
            tc.tile_pool(name="psS", bufs=2, space="PSUM") as psS,
            tc.tile_pool(name="psb", bufs=2, space="PSUM") as psbp,
        ):
            # ---- resident constants. ones2 (needed by the first stats
            # matmul) + onesR ride Sync; weights/biases ride Scalar behind
            # the chunk-0 stats tensor so the rstd chain starts ASAP. ----
            ones2 = cpool.tile([128, 2, 64], F8, tag="ones2")
            nc.sync.dma_start(ones2[:], ones2_d[:])
            onesR = cpool.tile([1, 128], BF16, tag="onesR")
            nc.sync.dma_start(onesR[:], onesr_d[:])
            bseed = cpool.tile([1, 1], F32, tag="bseed")
            nc.sync.dma_start(bseed[:], cst_d[:])
            ones1 = cpool.tile([128, 1], BF16, tag="ones1")
            nc.sync.dma_start(ones1[:], ones1_d[:])
            wg_sb = cpool.tile([128, KT, OH], BF16, tag="wg")
            wc_sb = cpool.tile([128, KT, OH], BF16, tag="wc")
            bg_sb = cpool.tile([128, OT], F32, tag="bg")
            bgn_sb = cpool.tile([128, OT], F32, tag="bgn")
            bc_sb = cpool.tile([128, OT], F32, tag="bc")

            def warmup(n):
                # keep the PE busy from t~3us (before any DMA data can land:
                # the hardware DMA pipe has a ~9us cold-start) so the HAM
                # clock is at full rate when real matmuls start. The source
                # is a memzero'd SBUF tile - no DMA dependency. The PSUM
                # target shares the psbR bank; bcast(0) waits via WAW.
                warm_w = cpool.tile([1, CHUNK], BF16, tag="warm_w")
                nc.scalar.memzero(warm_w[:])
                psw = psbp.tile([128, CHUNK], F32, tag="psbR", bufs=1, name="psw")
                for _ in range(n):
                    nc.tensor.matmul(
                        psw[:], warm_w[:, 0:128], warm_w[:], start=True, stop=True
                    )

            def load_w(half):
                lo, hi = half * 256, half * 256 + 256
                nc.sync.dma_start(wg_sb[:, :, lo:hi], wg_d[:, :, lo:hi])
                nc.sync.dma_start(wc_sb[:, :, lo:hi], wc_d[:, :, lo:hi])

            def load_biases():
                nc.scalar.dma_start(bg_sb[:], bg_d[:])
                nc.scalar.dma_start(bgn_sb[:], bgn_d[:])
                nc.scalar.dma_start(bc_sb[:], bc_d[:])

            h_prev = [None] * 2
            h_pair = [None] * 2
            xc_t = [None] * N_CHUNKS     # raw bf16 x chunk (GEMM rhs + residual)
            xs_t = [None] * N_CHUNKS     # packed fp8 [x; x^2] chunk
            st_t = [None] * N_CHUNKS     # stats PSUM
            y1_t = [None] * N_CHUNKS     # rstd row (bf16) per chunk

            def load_x(i, split=1):
                xc = xpool.tile([128, KT, CHUNK], BF16, tag="xc")
                step = KT // split
                for j in range(0, KT, step):
                    nc.sync.dma_start(xc[:, j : j + step], xT_d[i, :, j : j + step])
                xc_t[i] = xc

            def load_xs(i, split=1):
                xs = stpool.tile([128, KT, 2, CHUNK], F8, tag="xs")
                step = KT // split
                for j in range(0, KT, step):
                    nc.scalar.dma_start(xs[:, j : j + step], xs_d[i, :, j : j + step])
                xs_t[i] = xs

            def stats0_bf16():
                # chunk 0 computes stats from the bf16 GEMM x directly -
                # squares on Scalar+GpSimd, 16 bf16 ones-matmuls - so the
                # startup critical path never waits for the fp8 stats DMA.
                xc = xc_t[0]
                xsq = xnpool.tile([128, KT, CHUNK], BF16, tag="xsq", bufs=1)
                for k in range(KT):
                    if k < 4:
                        nc.scalar.activation(xsq[:, k, :], xc[:, k, :], AF.Square)
                    else:
                        nc.vector.tensor_mul(xsq[:, k, :], xc[:, k, :], xc[:, k, :])
                st = psS.tile([64, CHUNK], F32, tag="st", name="st0")
                for k in range(KT):
                    nc.tensor.matmul(
                        st[0:1, :], ones1[:], xc[:, k, :],
                        start=(k == 0), stop=(k == KT - 1),
                    )
                for k in range(KT):
                    nc.tensor.matmul(
                        st[32:33, :], ones1[:], xsq[:, k, :],
                        start=(k == 0), stop=(k == KT - 1),
                    )
                st_t[0] = st

            def stats_mm(i):
                # DoubleRow fp8: one matmul per k-tile reduces BOTH x (row 0)
                # and x^2 (row 32) over its 128 partitions; PSUM accumulates
                # across the 8 k-tiles. Row 32 so downstream PSUM reads are
                # 32-partition aligned; other lhsT cols are zero padding.
                xs = xs_t[i]
                st = psS.tile([64, CHUNK], F32, tag="st")
                for k in range(KT):
                    nc.tensor.matmul(
                        st[:], ones2[:], xs[:, k, :, :],
                        start=(k == 0), stop=(k == KT - 1),
                        perf_mode=PM.DoubleRow,
                    )
                st_t[i] = st

            def stats_tail(i):
                """rstd via Square-seed + one Newton step; broadcast via PE."""
                st = st_t[i]
                mu2 = spool.tile([1, CHUNK], F32, tag="mu2")
                nc.scalar.activation(mu2[:], st[0:1, :], AF.Square, scale=1.0 / H)
                v = spool.tile([1, CHUNK], F32, tag="v")
                nc.vector.scalar_tensor_tensor(
                    v[:], st[32:33, :], 1.0 / H, mu2[:], OP.mult, OP.subtract
                )
                y0 = spool.tile([1, CHUNK], F32, tag="y0")
                nc.scalar.activation(y0[:], v[:], AF.Square, bias=bseed[:], scale=S_SEED)
                u = spool.tile([1, CHUNK], F32, tag="u")
                nc.scalar.activation(u[:], y0[:], AF.Square)
                z2 = spool.tile([1, CHUNK], F32, tag="z2")
                nc.vector.tensor_mul(z2[:], u[:], v[:])
                g = spool.tile([1, CHUNK], F32, tag="g")
                nc.vector.tensor_scalar(g[:], z2[:], -0.5, 1.5, OP.mult, OP.add)
                y1 = spool.tile([1, CHUNK], BF16, tag="y1")
                with nc.allow_low_precision(reason="bf16 rstd for bf16 GEMM prescale"):
                    nc.vector.tensor_mul(y1[:], g[:], y0[:])
                y1_t[i] = y1

            def xn_make(i):
                psb = psbp.tile([128, CHUNK], F32, tag="psbR", bufs=1)
                nc.tensor.matmul(psb[:], onesR[:], y1_t[i][:], start=True, stop=True)
                rstdB = spool.tile([128, CHUNK], BF16, tag="rstdB")
                with nc.allow_low_precision(reason="bf16 rstd broadcast"):
                    nc.vector.tensor_scalar_mul(rstdB[:], psb[:], 1.0)
                xc = xc_t[i]
                xn = xnpool.tile([128, KT, CHUNK], BF16, tag="xn")
                for k in range(KT):
                    nc.vector.tensor_mul(xn[:, k, :], xc[:, k, :], rstdB[:])
                return xn

            def gemm_o(i, o, xn):
                og = o * 128
                pg = psA.tile([128, CHUNK], F32, tag="pg")
                for k in range(KT):
                    nc.tensor.matmul(
                        pg[:], wg_sb[:, k, og : og + 128], xn[:, k, :],
                        start=(k == 0), stop=(k == KT - 1),
                    )
                pc = psB.tile([128, CHUNK], F32, tag="pc")
                for k in range(KT):
                    nc.tensor.matmul(
                        pc[:], wc_sb[:, k, og : og + 128], xn[:, k, :],
                        start=(k == 0), stop=(k == KT - 1),
                    )

                with nc.allow_low_precision(reason="bf16 gates/candidate"):
                    z = wpool.tile([128, CHUNK], BF16, tag="z")
                    nc.scalar.activation(z[:], pg[:], AF.Sigmoid, bias=bg_sb[:, o : o + 1])
                    # a = 1 - z = sigmoid(-(pre + bg)) -- independent of z
                    a = wpool.tile([128, CHUNK], BF16, tag="a")
                    nc.scalar.activation(
                        a[:], pg[:], AF.Sigmoid, bias=bgn_sb[:, o : o + 1], scale=-1.0
                    )
                    cpb = wpool.tile([128, CHUNK], BF16, tag="cpb")
                    nc.scalar.add(cpb[:], pc[:], bc_sb[:, o : o + 1])
                bsc = wpool.tile([128, CHUNK], BF16, tag="bsc")
                nc.vector.tensor_mul(bsc[:], cpb[:], z[:])

                pair, j = divmod(o, 2)
                if j == 0:
                    h_pair[pair] = hpool.tile(
                        [128, 2, CHUNK], BF16, tag=f"hp{pair}", name=f"hp{pair}"
                    )
                h = h_pair[pair]
                init = 0.0 if i == 0 else h_prev[pair][:, j, CHUNK - 1 : CHUNK]
                nc.vector.tensor_tensor_scan(
                    h[:, j, :], a[:], bsc[:], init, OP.mult, OP.add
                )
                if i == N_CHUNKS - 1:
                    # final chunk: residual on DVE right behind the scan (no
                    # cross-engine hop, no slow GpSimd op on the drain path)
                    ot = wpool.tile([128, CHUNK], F32, tag="otl", name=f"otl{o}")
                    nc.vector.tensor_add(ot[:], h[:, j, :], xc_t[i][:, o, :])
                    nc.sync.dma_start(out_d[i, o], ot[:])
                elif j == 1:
                    h_prev[pair] = h
                    ot = wpool.tile([128, 2, CHUNK], F32, tag=f"ot{pair}")
                    nc.gpsimd.tensor_add(
                        ot[:], h[:], xc_t[i][:, 2 * pair : 2 * pair + 2, :]
                    )
                    nc.sync.dma_start(out_d[i, 2 * pair], ot[:, 0, :])
                    nc.sync.dma_start(out_d[i, 2 * pair + 1], ot[:, 1, :])

            # ---- software pipeline, stats run two chunks ahead: during
            # chunk i the PE interleaves stats matmuls for i+2 and the rstd
            # broadcast for i+1; the DVE prescale for i+1 runs mid-chunk so
            # chunk boundaries never wait on the rstd chain.
            # Startup: chunk 0 stats come from the bf16 x (no fp8 DMA on the
            # critical path); weights stream in column halves interleaved
            # with the x chunks; the scalar queue gets few enough doorbells
            # that the DGE ring never backpressures ACT compute. ----
            warmup(26)
            load_x(0, split=4)
            load_w(0)
            load_xs(1)
            load_biases()
            load_w(1)
            load_x(1)
            load_xs(2)
            stats0_bf16()
            stats_tail(0)
            stats_mm(1)
            stats_tail(1)
            xn = xn_make(0)
            for i in range(N_CHUNKS):
                nxt = i + 1 < N_CHUNKS
                if i + 3 < N_CHUNKS:
                    load_xs(i + 3)
                if i + 2 < N_CHUNKS:
                    load_x(i + 2)
                gemm_o(i, 0, xn)
                if nxt:
                    xn_next = xn_make(i + 1)
                gemm_o(i, 1, xn)
                if i + 2 < N_CHUNKS:
                    stats_mm(i + 2)
                gemm_o(i, 2, xn)
                if i + 2 < N_CHUNKS:
                    stats_tail(i + 2)
                gemm_o(i, 3, xn)
                if nxt:
                    xn = xn_next

    nc.compile()
    return nc


def _prep_weights(gamma, beta, Wg, bg, Wc, bc, ohalf):
    """Host-side weight folding for one output half.

    The h-rows of the weights (and of xT, see kernel()) are rolled so this
    half's own output channels come first: the device residual then always
    reads x rows at k-tiles 0..OT-1 with one shared program across cores.

    The LN mean-subtraction folds exactly into the weights: subtracting each
    output row's mean over h makes sum_h W''[o,h]*x[h] == sum_h W[o,h]*(x[h]-mu).
    """
    o0 = ohalf * OH
    perm = np.roll(np.arange(H), -o0)  # identity for half 0, swap halves for 1
    Wg_h = Wg[o0 : o0 + OH]          # [OH, H]
    Wc_h = Wc[o0 : o0 + OH]
    # lhsT layout [h, o], gamma folded into rows (h), rows permuted like xT
    wg_eff = ((Wg_h * gamma[None, :]).T)[perm].astype(np.float32)   # [H, OH]
    wc_eff = ((Wc_h * gamma[None, :]).T)[perm].astype(np.float32)
    wg_eff -= wg_eff.mean(axis=0, keepdims=True)
    wc_eff -= wc_eff.mean(axis=0, keepdims=True)
    bg_eff = (bg[o0 : o0 + OH] + Wg_h @ beta).astype(np.float32)
    bc_eff = (bc[o0 : o0 + OH] + Wc_h @ beta).astype(np.float32)

    def tile_w(w):  # [H, OH] -> [128, KT, OH]
        return np.ascontiguousarray(w.reshape(KT, 128, OH).transpose(1, 0, 2))

    # sum(x) lands at out partition 0, sum(x^2) at partition 32 (PSUM reads
    # by other engines must start at a 32-aligned partition)
    ones2 = np.zeros((128, 2, 64), dtype=NP8)
    ones2[:, 0, 0] = 1.0
    ones2[:, 1, 32] = 1.0

    return {
        "wg": tile_w(wg_eff.astype(BF)),
        "wc": tile_w(wc_eff.astype(BF)),
        "bg": np.ascontiguousarray(bg_eff.reshape(OT, 128).T),
        "bgn": np.ascontiguousarray(-bg_eff.reshape(OT, 128).T),
        "bc": np.ascontiguousarray(bc_eff.reshape(OT, 128).T),
        "ones2": ones2,
        "onesr": np.ones((1, 128), dtype=BF),
        "ones1": np.ones((128, 1), dtype=BF),
        "cst": np.full((1, 1), B_SEED, dtype=np.float32),
    }


def kernel(x, gamma, beta, Wg, bg, Wc, bc):
    x = np.asarray(x, dtype=np.float32)
    gamma = np.asarray(gamma, dtype=np.float32)
    beta = np.asarray(beta, dtype=np.float32)
    Wg = np.asarray(Wg, dtype=np.float32)
    bg = np.asarray(bg, dtype=np.float32)
    Wc = np.asarray(Wc, dtype=np.float32)
    bc = np.asarray(bc, dtype=np.float32)

    if "nc" not in _CACHE:
        _CACHE["nc"] = _build()
    nc = _CACHE["nc"]

    xT = [np.ascontiguousarray(x[b].T) for b in range(B)]  # [H, T] each
    halves = [_prep_weights(gamma, beta, Wg, bg, Wc, bc, p) for p in range(2)]

    def tile_x(xr):  # [H, T] -> [chunks, 128, KT, CHUNK]
        return xr.reshape(KT, 128, N_CHUNKS, CHUNK).transpose(2, 1, 0, 3)

    # packed fp8 stats tensor, shared by both halves of a batch (the sums
    # over h are invariant to the row roll)
    xstat = []
    for b in range(B):
        x8 = tile_x(xT[b].astype(NP8))
        xsq8 = tile_x((xT[b] * xT[b]).astype(NP8))
        xstat.append(
            np.ascontiguousarray(np.stack([x8, xsq8], axis=3))
        )  # [chunks, 128, KT, 2, CHUNK]

    in_maps = []
    for c in range(N_CORES):
        b, p = divmod(c, 2)
        m = dict(halves[p])
        # roll h-rows to match the weight-row permutation for this half
        xr = xT[b] if p == 0 else np.roll(xT[b], -OH, axis=0)
        m["xT"] = np.ascontiguousarray(tile_x(xr.astype(BF)))
        m["xs"] = xstat[b]
        in_maps.append(m)

    trace = bool(int(os.environ.get("MINGRU_TRACE", "0")))
    kwargs = {}
    if trace:
        tmpdir = os.environ.get("MINGRU_TRACE_DIR") or None
        kwargs = dict(trace=True, tmpdir=tmpdir)
    res = run_bass_kernel_spmd(nc, in_maps, core_ids=list(range(N_CORES)), **kwargs)
    if trace:
        _CACHE["last_results"] = res

    out = np.empty((B, T, H), dtype=np.float32)
    for c in range(N_CORES):
        b, p = divmod(c, 2)
        # [chunks, OT, 128, CHUNK] -> [OH, T] -> [T, OH]
        oT = res.results[c]["outT"].transpose(1, 2, 0, 3).reshape(OH, T)
        out[b, :, p * OH : (p + 1) * OH] = oT.T
    return out
